# revision 32
# baseline (speedup 1.0000x reference)
"""HTSK fuzzy-system kernel for Trainium2 (Bass/Tile), 8-core data-parallel.

Math (per batch row b):
  S     = H/sigma^2 + EPS                          (D,R)
  m     = mean_d(-(X_bd - C_dr)^2 * S_dr)          (B,R)
        = X^2 @ A + X @ Bm + 1*K2                  (matmul expansion)
  e_n   = softmax_r(m)                             (normalized firing)
  G     = X @ Wt',  Wt'[d, o*R+r] = W[r*D+d, o]    (o-major columns)
  out   = sum_r e_n[b,r]*G[b,o,r]  +  e_n @ (W2 + 1 b^T)

Schedule: per 128-row tile, PE streams G into PSUM in 1024-col groups,
ACT evicts psum->bf16, DVE (2x mode; o-major keeps unit stride under the
e-broadcast) multiplies o<40 by e_n, Pool multiplies o>=40 (large
contiguous op only - Pool is slow on small/strided work), DVE runs a
3-level halving tree + segmented tensor_reduce. Phase-1 (softmax) of
tile t+2 is software-pipelined behind tile t's G phase. All DMAs issue
from the sync engine; constants ride in one packed tensor.

Sharding: batch B=4096 split 512 rows per core; weights replicated.
"""
import sys
import types
from contextlib import ExitStack

import numpy as np

sys.path.insert(0, "/opt/trn_rl_repo")

# NTFF profile-hook registry: trn_boot §6 sets it at jax init, concourse
# bass_utils reads it when trace=True. The container's antenv package lacks
# this submodule, so provide it before anything imports jax/concourse.
if "antenv.axon_hooks" not in sys.modules:
    _ah = types.ModuleType("antenv.axon_hooks")
    _ah._hook = None

    def _set_hook(hook):
        _ah._hook = hook

    def _get_hook():
        return _ah._hook

    _ah.set_axon_ntff_profile_hook = _set_hook
    _ah.get_axon_ntff_profile_hook = _get_hook
    sys.modules["antenv.axon_hooks"] = _ah

import ml_dtypes  # noqa: E402
import concourse.bass as bass  # noqa: E402
import concourse.bacc as bacc  # noqa: E402
import concourse.tile as tile  # noqa: E402
from concourse import mybir  # noqa: E402
from concourse import bass_utils  # noqa: E402
from concourse.masks import make_identity  # noqa: E402

H = 0.5
EPS = 1e-8
B, D, R, O = 4096, 256, 128, 64
NCORES = 8
BL = B // NCORES          # 512 batch rows per core
NT = BL // 128            # 4 partition tiles per core
RO = R * O                # 8192
NG = 8                    # 1024-col (8-o) G groups per tile
O_MUL = 40                # DVE multiplies o < O_MUL, Pool the rest
F32 = mybir.dt.float32
BF16 = mybir.dt.bfloat16

_CACHE = {}


def _build():
    nc = bacc.Bacc("TRN2", target_bir_lowering=False, debug=False)
    X = nc.dram_tensor("X", [BL, D], BF16, kind="ExternalInput")
    # packed consts: [A_c0 | A_c1 | Bm_c0 | Bm_c1 | W2p | K2row] columns
    PK = nc.dram_tensor("PK", [128, 5 * R + O], BF16, kind="ExternalInput")
    Wt = nc.dram_tensor("Wt", [D, RO], BF16, kind="ExternalInput")
    out = nc.dram_tensor("out", [BL, O], F32, kind="ExternalOutput")

    with tile.TileContext(nc) as tc, ExitStack() as ctx:
        consts = ctx.enter_context(tc.tile_pool(name="consts", bufs=1))
        ph1p = ctx.enter_context(tc.tile_pool(name="ph1", bufs=2))
        epool = ctx.enter_context(tc.tile_pool(name="ep", bufs=3))
        gsbp = ctx.enter_context(tc.tile_pool(name="gsb", bufs=2))
        gmwp = ctx.enter_context(tc.tile_pool(name="gmw", bufs=2))
        trp = ctx.enter_context(tc.tile_pool(name="tr", bufs=2))
        osbp = ctx.enter_context(tc.tile_pool(name="osb", bufs=2))
        ps_m = ctx.enter_context(tc.tile_pool(name="ps_m", bufs=1, space="PSUM"))
        ps_a = ctx.enter_context(tc.tile_pool(name="ps_a", bufs=1, space="PSUM"))
        ps_g = ctx.enter_context(tc.tile_pool(name="ps_g", bufs=3, space="PSUM"))

        # ---- constants + X, all on the sync HWDGE queue; Wt streams in
        # G-group order behind them ----
        identB = consts.tile([128, 128], BF16, tag="idb")
        make_identity(nc, identB)
        ones_sb = consts.tile([1, 128], BF16, tag="ones")
        nc.vector.memset(ones_sb, 1.0)
        # exactly 4 DMA issues per HWDGE queue: a 5th recycles an earlier
        # transfer's semaphore and poisons its consumers with a false wait
        pk_sb = consts.tile([128, 5 * R + O], BF16, tag="pk")
        nc.sync.dma_start(out=pk_sb, in_=PK[:, :])
        xall = consts.tile([128, NT, D], BF16, tag="xall")
        nc.scalar.dma_start(
            out=xall, in_=X[:, :].rearrange("(t p) d -> p t d", t=NT)
        )
        wt_t = [[None] * 3 for _ in range(2)]
        SPL = [0, 2048, 5120, 8192]
        for c in range(2):
            eng = nc.sync if c == 0 else nc.scalar
            for k in range(3):
                w_ = consts.tile([128, SPL[k + 1] - SPL[k]], BF16,
                                 tag=f"wt{c}{k}", name=f"wt{c}{k}")
                eng.dma_start(
                    out=w_[:, :],
                    in_=Wt[c * 128:(c + 1) * 128, SPL[k]:SPL[k + 1]],
                )
                wt_t[c][k] = w_

        def wt_chunk(c, g, h):
            k = 0 if g < 2 else (1 if g < 5 else 2)
            off = g * 1024 + h * 512 - SPL[k]
            return wt_t[c][k][:, off:off + 512]

        k2_sb = pk_sb[0:1, 4 * R + O:5 * R + O]
        a_rhs = [pk_sb[:, 0:R], pk_sb[:, R:2 * R]]
        bm_rhs = [pk_sb[:, 2 * R:3 * R], pk_sb[:, 3 * R:4 * R]]
        w2p_sb = pk_sb[:, 4 * R:4 * R + O]

        def ph1_a(t):
            """Transpose X tile, square, membership-logit matmuls."""
            xtT = ps_a.tile([128, 2, 128], BF16, tag="aux", name=f"xtT{t}")
            for c in range(2):
                nc.tensor.transpose(
                    xtT[:, c, :], xall[:, t, c * 128:(c + 1) * 128], identB
                )
            xTb = ph1p.tile([128, 2, 128], BF16, tag="xTb", name=f"xTb{t}")
            x2Tb = ph1p.tile([128, 2, 128], BF16, tag="x2Tb", name=f"x2Tb{t}")
            nc.vector.tensor_copy(xTb, xtT)
            # square on DVE (self-multiply, 2x) to unload the ACT engine
            nc.vector.tensor_mul(x2Tb, xTb, xTb)
            m_ps = ps_m.tile([128, R], F32, tag="m", name=f"m{t}")
            nc.tensor.matmul(m_ps, lhsT=x2Tb[:, 0, :], rhs=a_rhs[0],
                             start=True, stop=False)
            nc.tensor.matmul(m_ps, lhsT=x2Tb[:, 1, :], rhs=a_rhs[1],
                             start=False, stop=False)
            nc.tensor.matmul(m_ps, lhsT=xTb[:, 0, :], rhs=bm_rhs[0],
                             start=False, stop=False)
            nc.tensor.matmul(m_ps, lhsT=xTb[:, 1, :], rhs=bm_rhs[1],
                             start=False, stop=False)
            nc.tensor.matmul(m_ps, lhsT=ones_sb, rhs=k2_sb,
                             start=False, stop=True)
            return xTb, m_ps

        def ph1_b(t, m_ps):
            """Unnormalized softmax weights e = exp(m - max), s = sum e."""
            e_raw = epool.tile([128, R], BF16, tag="eraw", name=f"eraw{t}")
            s_ = ph1p.tile([128, 1], F32, tag="s", name=f"s{t}")
            # m <= 0 always, so exp cannot overflow; the softmax scale is
            # absorbed by the final 1/s normalization
            nc.scalar.activation(e_raw, m_ps, mybir.ActivationFunctionType.Exp,
                                 bias=0.0, scale=1.0, accum_out=s_)
            rs = epool.tile([128, 1], F32, tag="rs", name=f"rs{t}")
            nc.vector.reciprocal(rs, s_)
            return e_raw, rs

        def ph1_c(t, e_raw):
            """out2 = e @ (W2 + 1 b^T), unnormalized."""
            eT = ps_a.tile([128, 128], BF16, tag="aux", name=f"eT{t}")
            nc.tensor.transpose(eT, e_raw, identB)
            eTs = ph1p.tile([128, 128], BF16, tag="eTs", name=f"eTs{t}")
            nc.vector.tensor_copy(eTs, eT)
            o2p = ps_a.tile([128, O], F32, tag="aux", name=f"o2{t}")
            nc.tensor.matmul(o2p, lhsT=eTs, rhs=w2p_sb, start=True, stop=True)
            o2s = epool.tile([128, O], F32, tag="o2s", name=f"o2s{t}")
            nc.vector.tensor_copy(o2s, o2p)
            return o2s

        def ph2(t, ctx_t, pend):
            """G matmul stream + e-weighting + partial tree, with the
            next tiles' ph1 stages software-pipelined into the loop."""
            xTb, e_raw, rs, o2s = ctx_t[t]
            gsb = gsbp.tile([128, RO], BF16, tag="gsb", name=f"gsb{t}")
            gmw = gmwp.tile([128, O, R], BF16, tag="gmw", name=f"gmw{t}")
            ebc16 = e_raw.rearrange("p r -> p () r").broadcast_to((128, 16, R))
            ebc32 = e_raw.rearrange("p r -> p () r").broadcast_to((128, 32, R))
            ebc = e_raw.rearrange("p r -> p () r").broadcast_to((128, 8, R))
            tr64 = trp.tile([128, O, 64], BF16, tag="t64", name=f"t64_{t}")
            tr32 = trp.tile([128, O, 32], BF16, tag="t32", name=f"t32_{t}")
            tr16 = trp.tile([128, O, 16], BF16, tag="t16", name=f"t16_{t}")
            for g in range(NG):
                gt = ps_g.tile([128, 1024], F32, tag="g", name=f"g_{t}_{g}")
                for h in range(2):
                    for c in range(2):
                        nc.tensor.matmul(
                            gt[:, h * 512:(h + 1) * 512],
                            lhsT=xTb[:, c, :],
                            rhs=wt_chunk(c, g, h),
                            start=(c == 0), stop=(c == 1),
                        )
                if t == 0 and g < 3:
                    # fuse evict+weighting for the first groups: DVE
                    # multiplies straight out of PSUM (1x) so its pipeline
                    # starts early in the DMA-paced head; later groups use
                    # the cheaper ACT-evict + 2x-mul path (ACT is idle here)
                    gv = gt.rearrange("p (o r) -> p o r", r=R)
                    nc.vector.tensor_mul(gmw[:, 8 * g:8 * (g + 1), :], gv, ebc )
                else:
                    nc.scalar.copy(gsb[:, g * 1024:(g + 1) * 1024], gt)
                if t == 0:
                    if g == 4:
                        gv = gsb[:, 3072:5120].rearrange("p (o r) -> p o r", r=R)
                        nc.vector.tensor_mul(gmw[:, 24:40, :], gv, ebc16)
                    if g == 6:
                        gv = gsb[:, 5120:7168].rearrange("p (o r) -> p o r", r=R)
                        nc.vector.tensor_mul(gmw[:, 40:56, :], gv, ebc16)
                    if g == 7:
                        gv = gsb[:, 7168:8192].rearrange("p (o r) -> p o r", r=R)
                        nc.vector.tensor_mul(gmw[:, 56:O, :], gv, ebc)
                fine = (t == NT - 1)
                if g == 3 and not fine and t != 0:
                    gv = gsb[:, 0:4096].rearrange("p (o r) -> p o r", r=R)
                    nc.vector.tensor_mul(gmw[:, 0:32, :], gv, ebc32)
                if g == 7 and not fine and t != 0:
                    gv = gsb[:, 4096:8192].rearrange("p (o r) -> p o r", r=R)
                    nc.vector.tensor_mul(gmw[:, 32:O, :], gv, ebc32)
                if g == 3 and not fine and t != 0:
                    nc.vector.tensor_add(
                        tr64[:, 0:32, :], gmw[:, 0:32, 0:64], gmw[:, 0:32, 64:R]
                    )
                if g == 5 and not fine and t != 0:
                    nc.vector.tensor_add(
                        tr32[:, 0:32, :], tr64[:, 0:32, 0:32], tr64[:, 0:32, 32:64]
                    )
                if g == 7 and not fine and t != 0:
                    nc.vector.tensor_add(
                        tr16[:, 0:32, :], tr32[:, 0:32, 0:16], tr32[:, 0:32, 16:32]
                    )
                if fine:
                    if g % 2 == 1:
                        k = g // 2
                        gv = gsb[:, k * 2048:(k + 1) * 2048].rearrange(
                            "p (o r) -> p o r", r=R
                        )
                        nc.vector.tensor_mul(
                            gmw[:, 16 * k:16 * (k + 1), :], gv, ebc16
                        )
                    if g == 3:
                        nc.vector.tensor_add(
                            tr64[:, 0:32, :], gmw[:, 0:32, 0:64], gmw[:, 0:32, 64:R]
                        )
                    if g == 5:
                        nc.vector.tensor_add(
                            tr32[:, 0:32, :], tr64[:, 0:32, 0:32],
                            tr64[:, 0:32, 32:64]
                        )
                        nc.vector.tensor_add(
                            tr64[:, 32:48, :], gmw[:, 32:48, 0:64],
                            gmw[:, 32:48, 64:R]
                        )
                        nc.vector.tensor_add(
                            tr32[:, 32:48, :], tr64[:, 32:48, 0:32],
                            tr64[:, 32:48, 32:64]
                        )
                    if g == 7:
                        nc.vector.tensor_add(
                            tr16[:, 0:32, :], tr32[:, 0:32, 0:16],
                            tr32[:, 0:32, 16:32]
                        )
                        nc.vector.tensor_add(
                            tr16[:, 32:48, :], tr32[:, 32:48, 0:16],
                            tr32[:, 32:48, 16:32]
                        )
                # stage C of tile t+1's ph1 (out2 matmul) early in this
                # loop: its exp() finished last tile, so no PE stall here
                if g == 1 and pend.get(t + 1) is not None:
                    xTb1, e1, rs1 = pend.pop(t + 1)
                    ctx_t.append((xTb1, e1, rs1, ph1_c(t + 1, e1)))
                # stages A+B of tile t+2's ph1 after all G matmuls are
                # emitted, keeping the PE G-stream contiguous for the
                # p-state ramp
                if g == 7 and t + 2 < NT:
                    a = ph1_a(t + 2)
                    b = ph1_b(t + 2, a[1])
                    pend[t + 2] = (a[0], b[0], b[1])
            return gsb, gmw, tr64, tr32, tr16

        def trees(t, gmw, tr64, tr32, tr16, rs, o2s):
            """Remaining tree + combine + normalize + store."""
            osb = osbp.tile([128, O], F32, tag="osb", name=f"osb{t}")
            lo = 48 if t == NT - 1 else (0 if t == 0 else 32)
            nc.vector.tensor_add(
                tr64[:, lo:O, :], gmw[:, lo:O, 0:64], gmw[:, lo:O, 64:R]
            )
            nc.vector.tensor_add(
                tr32[:, lo:O, :], tr64[:, lo:O, 0:32], tr64[:, lo:O, 32:64]
            )
            nc.vector.tensor_add(
                tr16[:, lo:O, :], tr32[:, lo:O, 0:16], tr32[:, lo:O, 16:32]
            )
            tr8 = trp.tile([128, O, 8], BF16, tag="t8", name=f"t8_{t}")
            red = trp.tile([128, O], F32, tag="red", name=f"red{t}")
            if t == NT - 1:
                nc.vector.tensor_add(
                    tr8[:, 0:32, :], tr16[:, 0:32, 0:8], tr16[:, 0:32, 8:16]
                )
                nc.vector.reduce_sum(red[:, 0:32], tr8[:, 0:32, :],
                                     axis=mybir.AxisListType.X)
                nc.vector.tensor_add(osb[:, 0:32], red[:, 0:32], o2s[:, 0:32])
                nc.vector.tensor_add(
                    tr8[:, 32:O, :], tr16[:, 32:O, 0:8], tr16[:, 32:O, 8:16]
                )
                nc.vector.reduce_sum(red[:, 32:O], tr8[:, 32:O, :],
                                     axis=mybir.AxisListType.X)
                nc.vector.tensor_add(osb[:, 32:O], red[:, 32:O], o2s[:, 32:O])
            else:
                nc.vector.tensor_add(tr8, tr16[:, :, 0:8], tr16[:, :, 8:16])
                nc.vector.reduce_sum(red, tr8, axis=mybir.AxisListType.X)
                nc.vector.tensor_add(osb, red, o2s)
            oscl = osbp.tile([128, O], F32, tag="oscl", name=f"oscl{t}")
            nc.scalar.activation(oscl, osb, mybir.ActivationFunctionType.Copy,
                                 scale=rs)
            nc.sync.dma_start(out=out[t * 128:(t + 1) * 128, :], in_=oscl)

        def ph1_full(t):
            xTb, m_ps = ph1_a(t)
            e_raw, rs = ph1_b(t, m_ps)
            o2s = ph1_c(t, e_raw)
            return (xTb, e_raw, rs, o2s)

        ctx_t = [ph1_full(0), ph1_full(1)]
        pend = {}
        for t in range(NT):
            gsb, gmw, tr64, tr32, tr16 = ph2(t, ctx_t, pend)
            trees(t, gmw, tr64, tr32, tr16, ctx_t[t][2], ctx_t[t][3])

    nc.finalize()
    return nc


def _get_nc():
    if "nc" not in _CACHE:
        _CACHE["nc"] = _build()
    return _CACHE["nc"]


def _host_prep(centers, sigmas, W, b):
    c64 = centers.astype(np.float64)
    S = (H / sigmas.astype(np.float64) ** 2) + EPS          # (D,R)
    A = (-S / D).astype(ml_dtypes.bfloat16)                  # X^2 coeff
    Bm = (2.0 * S * c64 / D).astype(ml_dtypes.bfloat16)      # X coeff
    K2 = (-(S * c64 * c64).sum(axis=0, keepdims=True) / D).astype(
        ml_dtypes.bfloat16
    )
    W1 = W[: D * R].reshape(R, D, O)
    # o-major columns: Wt[d, o*R + r] = W1[r, d, o]
    Wt = np.ascontiguousarray(W1.transpose(1, 2, 0).reshape(D, RO)).astype(
        ml_dtypes.bfloat16
    )
    W2p = (W[D * R:].astype(np.float64) + b[None, :].astype(np.float64)).astype(
        ml_dtypes.bfloat16
    )
    # packed consts: [A_c0 | A_c1 | Bm_c0 | Bm_c1 | W2p | K2 bcast]
    K2b = np.broadcast_to(np.asarray(K2), (128, 128))
    PK = np.concatenate(
        [np.asarray(A[0:128]), np.asarray(A[128:256]),
         np.asarray(Bm[0:128]), np.asarray(Bm[128:256]), np.asarray(W2p),
         K2b],
        axis=1,
    ).astype(ml_dtypes.bfloat16)
    return np.ascontiguousarray(PK), Wt


def kernel(X, centers, sigmas, W, b):
    X = np.asarray(X, dtype=np.float32)
    centers = np.asarray(centers, dtype=np.float32)
    sigmas = np.asarray(sigmas, dtype=np.float32)
    W = np.asarray(W, dtype=np.float32)
    b = np.asarray(b, dtype=np.float32)

    PK, Wt = _host_prep(centers, sigmas, W, b)
    Xb = X.astype(ml_dtypes.bfloat16)
    nc = _get_nc()
    in_maps = [
        {
            "X": np.ascontiguousarray(Xb[k * BL:(k + 1) * BL]),
            "PK": PK, "Wt": Wt,
        }
        for k in range(NCORES)
    ]
    res = bass_utils.run_bass_kernel_spmd(nc, in_maps, core_ids=list(range(NCORES)))
    return np.concatenate([res.results[k]["out"] for k in range(NCORES)], axis=0)


# revision 34
# speedup vs baseline: 1.0104x; 1.0104x over previous
"""HTSK fuzzy-system kernel for Trainium2 (Bass/Tile), 8-core data-parallel.

Math (per batch row b):
  S     = H/sigma^2 + EPS                          (D,R)
  m     = mean_d(-(X_bd - C_dr)^2 * S_dr)          (B,R)
        = X^2 @ A + X @ Bm + 1*K2                  (matmul expansion)
  e_n   = softmax_r(m)                             (normalized firing)
  G     = X @ Wt',  Wt'[d, o*R+r] = W[r*D+d, o]    (o-major columns)
  out   = sum_r e_n[b,r]*G[b,o,r]  +  e_n @ (W2 + 1 b^T)

Schedule: per 128-row tile, PE streams G into PSUM in 1024-col groups,
ACT evicts psum->bf16, DVE (2x mode; o-major keeps unit stride under the
e-broadcast) multiplies o<40 by e_n, Pool multiplies o>=40 (large
contiguous op only - Pool is slow on small/strided work), DVE runs a
3-level halving tree + segmented tensor_reduce. Phase-1 (softmax) of
tile t+2 is software-pipelined behind tile t's G phase. All DMAs issue
from the sync engine; constants ride in one packed tensor.

Sharding: batch B=4096 split 512 rows per core; weights replicated.
"""
import sys
import types
from contextlib import ExitStack

import numpy as np

sys.path.insert(0, "/opt/trn_rl_repo")

# NTFF profile-hook registry: trn_boot §6 sets it at jax init, concourse
# bass_utils reads it when trace=True. The container's antenv package lacks
# this submodule, so provide it before anything imports jax/concourse.
if "antenv.axon_hooks" not in sys.modules:
    _ah = types.ModuleType("antenv.axon_hooks")
    _ah._hook = None

    def _set_hook(hook):
        _ah._hook = hook

    def _get_hook():
        return _ah._hook

    _ah.set_axon_ntff_profile_hook = _set_hook
    _ah.get_axon_ntff_profile_hook = _get_hook
    sys.modules["antenv.axon_hooks"] = _ah

import ml_dtypes  # noqa: E402
import concourse.bass as bass  # noqa: E402
import concourse.bacc as bacc  # noqa: E402
import concourse.tile as tile  # noqa: E402
from concourse import mybir  # noqa: E402
from concourse import bass_utils  # noqa: E402
from concourse.masks import make_identity  # noqa: E402

H = 0.5
EPS = 1e-8
B, D, R, O = 4096, 256, 128, 64
NCORES = 8
BL = B // NCORES          # 512 batch rows per core
NT = BL // 128            # 4 partition tiles per core
RO = R * O                # 8192
NG = 8                    # 1024-col (8-o) G groups per tile
O_MUL = 40                # DVE multiplies o < O_MUL, Pool the rest
F32 = mybir.dt.float32
BF16 = mybir.dt.bfloat16

_CACHE = {}


def _build():
    nc = bacc.Bacc("TRN2", target_bir_lowering=False, debug=False)
    X = nc.dram_tensor("X", [BL, D], BF16, kind="ExternalInput")
    # packed consts: [A_c0 | A_c1 | Bm_c0 | Bm_c1 | W2p | K2row] columns
    PK = nc.dram_tensor("PK", [128, 5 * R + O], BF16, kind="ExternalInput")
    Wt = nc.dram_tensor("Wt", [D, RO], BF16, kind="ExternalInput")
    out = nc.dram_tensor("out", [BL, O], F32, kind="ExternalOutput")

    with tile.TileContext(nc) as tc, ExitStack() as ctx:
        consts = ctx.enter_context(tc.tile_pool(name="consts", bufs=1))
        ph1p = ctx.enter_context(tc.tile_pool(name="ph1", bufs=2))
        epool = ctx.enter_context(tc.tile_pool(name="ep", bufs=3))
        gsbp = ctx.enter_context(tc.tile_pool(name="gsb", bufs=2))
        gmwp = ctx.enter_context(tc.tile_pool(name="gmw", bufs=2))
        trp = ctx.enter_context(tc.tile_pool(name="tr", bufs=2))
        osbp = ctx.enter_context(tc.tile_pool(name="osb", bufs=2))
        ps_m = ctx.enter_context(tc.tile_pool(name="ps_m", bufs=1, space="PSUM"))
        ps_a = ctx.enter_context(tc.tile_pool(name="ps_a", bufs=1, space="PSUM"))
        ps_g = ctx.enter_context(tc.tile_pool(name="ps_g", bufs=3, space="PSUM"))

        # ---- constants + X, all on the sync HWDGE queue; Wt streams in
        # G-group order behind them ----
        identB = consts.tile([128, 128], BF16, tag="idb")
        make_identity(nc, identB)
        ones_sb = consts.tile([1, 128], BF16, tag="ones")
        nc.vector.memset(ones_sb, 1.0)
        # exactly 4 DMA issues per HWDGE queue: a 5th recycles an earlier
        # transfer's semaphore and poisons its consumers with a false wait
        pk_sb = consts.tile([128, 5 * R + O], BF16, tag="pk")
        nc.sync.dma_start(out=pk_sb, in_=PK[:, :])
        xall = consts.tile([128, NT, D], BF16, tag="xall")
        nc.scalar.dma_start(
            out=xall, in_=X[:, :].rearrange("(t p) d -> p t d", t=NT)
        )
        wt_t = [[None] * 3 for _ in range(2)]
        SPL = [0, 2048, 5120, 8192]
        for c in range(2):
            eng = nc.sync if c == 0 else nc.scalar
            for k in range(3):
                w_ = consts.tile([128, SPL[k + 1] - SPL[k]], BF16,
                                 tag=f"wt{c}{k}", name=f"wt{c}{k}")
                eng.dma_start(
                    out=w_[:, :],
                    in_=Wt[c * 128:(c + 1) * 128, SPL[k]:SPL[k + 1]],
                )
                wt_t[c][k] = w_

        def wt_chunk(c, g, h):
            k = 0 if g < 2 else (1 if g < 5 else 2)
            off = g * 1024 + h * 512 - SPL[k]
            return wt_t[c][k][:, off:off + 512]

        k2_sb = pk_sb[0:1, 4 * R + O:5 * R + O]
        a_rhs = [pk_sb[:, 0:R], pk_sb[:, R:2 * R]]
        bm_rhs = [pk_sb[:, 2 * R:3 * R], pk_sb[:, 3 * R:4 * R]]
        w2p_sb = pk_sb[:, 4 * R:4 * R + O]

        def ph1_a(t):
            """Transpose X tile, square, membership-logit matmuls."""
            xtT = ps_a.tile([128, 2, 128], BF16, tag="aux", name=f"xtT{t}")
            for c in range(2):
                nc.tensor.transpose(
                    xtT[:, c, :], xall[:, t, c * 128:(c + 1) * 128], identB
                )
            xTb = ph1p.tile([128, 2, 128], BF16, tag="xTb", name=f"xTb{t}")
            x2Tb = ph1p.tile([128, 2, 128], BF16, tag="x2Tb", name=f"x2Tb{t}")
            nc.scalar.copy(xTb, xtT)
            # square on DVE (self-multiply, 2x) to unload the ACT engine
            nc.vector.tensor_mul(x2Tb, xTb, xTb)
            m_ps = ps_m.tile([128, R], F32, tag="m", name=f"m{t}")
            nc.tensor.matmul(m_ps, lhsT=x2Tb[:, 0, :], rhs=a_rhs[0],
                             start=True, stop=False)
            nc.tensor.matmul(m_ps, lhsT=x2Tb[:, 1, :], rhs=a_rhs[1],
                             start=False, stop=False)
            nc.tensor.matmul(m_ps, lhsT=xTb[:, 0, :], rhs=bm_rhs[0],
                             start=False, stop=False)
            nc.tensor.matmul(m_ps, lhsT=xTb[:, 1, :], rhs=bm_rhs[1],
                             start=False, stop=False)
            nc.tensor.matmul(m_ps, lhsT=ones_sb, rhs=k2_sb,
                             start=False, stop=True)
            return xTb, m_ps

        def ph1_b(t, m_ps):
            """Unnormalized softmax weights e = exp(m - max), s = sum e."""
            e_raw = epool.tile([128, R], BF16, tag="eraw", name=f"eraw{t}")
            s_ = ph1p.tile([128, 1], F32, tag="s", name=f"s{t}")
            # m <= 0 always, so exp cannot overflow; the softmax scale is
            # absorbed by the final 1/s normalization
            nc.scalar.activation(e_raw, m_ps, mybir.ActivationFunctionType.Exp,
                                 bias=0.0, scale=1.0, accum_out=s_)
            rs = epool.tile([128, 1], F32, tag="rs", name=f"rs{t}")
            nc.vector.reciprocal(rs, s_)
            return e_raw, rs

        def ph1_c(t, e_raw):
            """out2 = e @ (W2 + 1 b^T), unnormalized."""
            eT = ps_a.tile([128, 128], BF16, tag="aux", name=f"eT{t}")
            nc.tensor.transpose(eT, e_raw, identB)
            eTs = ph1p.tile([128, 128], BF16, tag="eTs", name=f"eTs{t}")
            nc.scalar.copy(eTs, eT)
            o2p = ps_a.tile([128, O], F32, tag="aux", name=f"o2{t}")
            nc.tensor.matmul(o2p, lhsT=eTs, rhs=w2p_sb, start=True, stop=True)
            o2s = epool.tile([128, O], F32, tag="o2s", name=f"o2s{t}")
            nc.vector.tensor_copy(o2s, o2p)
            return o2s

        def ph2(t, ctx_t, pend):
            """G matmul stream + e-weighting + partial tree, with the
            next tiles' ph1 stages software-pipelined into the loop."""
            xTb, e_raw, rs, o2s = ctx_t[t]
            gsb = gsbp.tile([128, RO], BF16, tag="gsb", name=f"gsb{t}")
            gmw = gmwp.tile([128, O, R], BF16, tag="gmw", name=f"gmw{t}")
            ebc16 = e_raw.rearrange("p r -> p () r").broadcast_to((128, 16, R))
            ebc32 = e_raw.rearrange("p r -> p () r").broadcast_to((128, 32, R))
            ebc = e_raw.rearrange("p r -> p () r").broadcast_to((128, 8, R))
            tr64 = trp.tile([128, O, 64], BF16, tag="t64", name=f"t64_{t}")
            tr32 = trp.tile([128, O, 32], BF16, tag="t32", name=f"t32_{t}")
            tr16 = trp.tile([128, O, 16], BF16, tag="t16", name=f"t16_{t}")
            for g in range(NG):
                gt = ps_g.tile([128, 1024], F32, tag="g", name=f"g_{t}_{g}")
                for h in range(2):
                    for c in range(2):
                        nc.tensor.matmul(
                            gt[:, h * 512:(h + 1) * 512],
                            lhsT=xTb[:, c, :],
                            rhs=wt_chunk(c, g, h),
                            start=(c == 0), stop=(c == 1),
                        )
                if (t == 0 and g < 3) or (t == NT - 1 and g >= 6):
                    # fused evict+weighting: DVE multiplies straight out of
                    # PSUM (1x). Tile 0 uses it to start the DVE pipeline
                    # early in the DMA-paced head; the last tile uses it for
                    # its final groups so the drain chain is not gated on
                    # the last ACT evictions
                    gv = gt.rearrange("p (o r) -> p o r", r=R)
                    nc.vector.tensor_mul(gmw[:, 8 * g:8 * (g + 1), :], gv, ebc )
                else:
                    nc.scalar.copy(gsb[:, g * 1024:(g + 1) * 1024], gt)
                if t == 0:
                    if g == 4:
                        gv = gsb[:, 3072:5120].rearrange("p (o r) -> p o r", r=R)
                        nc.vector.tensor_mul(gmw[:, 24:40, :], gv, ebc16)
                    if g == 6:
                        gv = gsb[:, 5120:7168].rearrange("p (o r) -> p o r", r=R)
                        nc.vector.tensor_mul(gmw[:, 40:56, :], gv, ebc16)
                    if g == 7:
                        gv = gsb[:, 7168:8192].rearrange("p (o r) -> p o r", r=R)
                        nc.vector.tensor_mul(gmw[:, 56:O, :], gv, ebc)
                fine = (t == NT - 1)
                if g == 3 and not fine and t != 0:
                    gv = gsb[:, 0:4096].rearrange("p (o r) -> p o r", r=R)
                    nc.vector.tensor_mul(gmw[:, 0:32, :], gv, ebc32)
                if g == 7 and not fine and t != 0:
                    gv = gsb[:, 4096:8192].rearrange("p (o r) -> p o r", r=R)
                    nc.vector.tensor_mul(gmw[:, 32:O, :], gv, ebc32)
                if g == 3 and not fine and t != 0:
                    nc.vector.tensor_add(
                        tr64[:, 0:32, :], gmw[:, 0:32, 0:64], gmw[:, 0:32, 64:R]
                    )
                if g == 5 and not fine and t != 0:
                    nc.vector.tensor_add(
                        tr32[:, 0:32, :], tr64[:, 0:32, 0:32], tr64[:, 0:32, 32:64]
                    )
                if g == 7 and not fine and t != 0:
                    nc.vector.tensor_add(
                        tr16[:, 0:32, :], tr32[:, 0:32, 0:16], tr32[:, 0:32, 16:32]
                    )
                if fine:
                    if g % 2 == 1 and g < 7:
                        k = g // 2
                        gv = gsb[:, k * 2048:(k + 1) * 2048].rearrange(
                            "p (o r) -> p o r", r=R
                        )
                        nc.vector.tensor_mul(
                            gmw[:, 16 * k:16 * (k + 1), :], gv, ebc16
                        )
                    if g == 3:
                        nc.vector.tensor_add(
                            tr64[:, 0:32, :], gmw[:, 0:32, 0:64], gmw[:, 0:32, 64:R]
                        )
                    if g == 5:
                        nc.vector.tensor_add(
                            tr32[:, 0:32, :], tr64[:, 0:32, 0:32],
                            tr64[:, 0:32, 32:64]
                        )
                        nc.vector.tensor_add(
                            tr64[:, 32:48, :], gmw[:, 32:48, 0:64],
                            gmw[:, 32:48, 64:R]
                        )
                        nc.vector.tensor_add(
                            tr32[:, 32:48, :], tr64[:, 32:48, 0:32],
                            tr64[:, 32:48, 32:64]
                        )
                    if g == 7:
                        nc.vector.tensor_add(
                            tr16[:, 0:32, :], tr32[:, 0:32, 0:16],
                            tr32[:, 0:32, 16:32]
                        )
                        nc.vector.tensor_add(
                            tr16[:, 32:48, :], tr32[:, 32:48, 0:16],
                            tr32[:, 32:48, 16:32]
                        )
                # stage C of tile t+1's ph1 (out2 matmul) early in this
                # loop: its exp() finished last tile, so no PE stall here
                if g == 1 and pend.get(t + 1) is not None:
                    xTb1, e1, rs1 = pend.pop(t + 1)
                    ctx_t.append((xTb1, e1, rs1, ph1_c(t + 1, e1)))
                # stages A+B of tile t+2's ph1 after all G matmuls are
                # emitted, keeping the PE G-stream contiguous for the
                # p-state ramp
                if g == 7 and t + 2 < NT:
                    a = ph1_a(t + 2)
                    b = ph1_b(t + 2, a[1])
                    pend[t + 2] = (a[0], b[0], b[1])
            return gsb, gmw, tr64, tr32, tr16

        def trees(t, gmw, tr64, tr32, tr16, rs, o2s):
            """Remaining tree + combine + normalize + store."""
            osb = osbp.tile([128, O], F32, tag="osb", name=f"osb{t}")
            lo = 48 if t == NT - 1 else (0 if t == 0 else 32)
            nc.vector.tensor_add(
                tr64[:, lo:O, :], gmw[:, lo:O, 0:64], gmw[:, lo:O, 64:R]
            )
            nc.vector.tensor_add(
                tr32[:, lo:O, :], tr64[:, lo:O, 0:32], tr64[:, lo:O, 32:64]
            )
            nc.vector.tensor_add(
                tr16[:, lo:O, :], tr32[:, lo:O, 0:16], tr32[:, lo:O, 16:32]
            )
            tr8 = trp.tile([128, O, 8], BF16, tag="t8", name=f"t8_{t}")
            red = trp.tile([128, O], F32, tag="red", name=f"red{t}")
            if t == NT - 1:
                nc.vector.tensor_add(
                    tr8[:, 0:32, :], tr16[:, 0:32, 0:8], tr16[:, 0:32, 8:16]
                )
                nc.vector.reduce_sum(red[:, 0:32], tr8[:, 0:32, :],
                                     axis=mybir.AxisListType.X)
                nc.vector.tensor_add(osb[:, 0:32], red[:, 0:32], o2s[:, 0:32])
                nc.vector.tensor_add(
                    tr8[:, 32:O, :], tr16[:, 32:O, 0:8], tr16[:, 32:O, 8:16]
                )
                nc.vector.reduce_sum(red[:, 32:O], tr8[:, 32:O, :],
                                     axis=mybir.AxisListType.X)
                nc.vector.tensor_add(osb[:, 32:O], red[:, 32:O], o2s[:, 32:O])
            else:
                nc.vector.tensor_add(tr8, tr16[:, :, 0:8], tr16[:, :, 8:16])
                nc.vector.reduce_sum(red, tr8, axis=mybir.AxisListType.X)
                nc.vector.tensor_add(osb, red, o2s)
            oscl = osbp.tile([128, O], F32, tag="oscl", name=f"oscl{t}")
            nc.scalar.activation(oscl, osb, mybir.ActivationFunctionType.Copy,
                                 scale=rs)
            nc.sync.dma_start(out=out[t * 128:(t + 1) * 128, :], in_=oscl)

        def ph1_full(t):
            xTb, m_ps = ph1_a(t)
            e_raw, rs = ph1_b(t, m_ps)
            o2s = ph1_c(t, e_raw)
            return (xTb, e_raw, rs, o2s)

        ctx_t = [ph1_full(0), ph1_full(1)]
        pend = {}
        for t in range(NT):
            gsb, gmw, tr64, tr32, tr16 = ph2(t, ctx_t, pend)
            trees(t, gmw, tr64, tr32, tr16, ctx_t[t][2], ctx_t[t][3])

    nc.finalize()
    return nc


def _get_nc():
    if "nc" not in _CACHE:
        _CACHE["nc"] = _build()
    return _CACHE["nc"]


def _host_prep(centers, sigmas, W, b):
    c64 = centers.astype(np.float64)
    S = (H / sigmas.astype(np.float64) ** 2) + EPS          # (D,R)
    A = (-S / D).astype(ml_dtypes.bfloat16)                  # X^2 coeff
    Bm = (2.0 * S * c64 / D).astype(ml_dtypes.bfloat16)      # X coeff
    K2 = (-(S * c64 * c64).sum(axis=0, keepdims=True) / D).astype(
        ml_dtypes.bfloat16
    )
    W1 = W[: D * R].reshape(R, D, O)
    # o-major columns: Wt[d, o*R + r] = W1[r, d, o]
    Wt = np.ascontiguousarray(W1.transpose(1, 2, 0).reshape(D, RO)).astype(
        ml_dtypes.bfloat16
    )
    W2p = (W[D * R:].astype(np.float64) + b[None, :].astype(np.float64)).astype(
        ml_dtypes.bfloat16
    )
    # packed consts: [A_c0 | A_c1 | Bm_c0 | Bm_c1 | W2p | K2 bcast]
    K2b = np.broadcast_to(np.asarray(K2), (128, 128))
    PK = np.concatenate(
        [np.asarray(A[0:128]), np.asarray(A[128:256]),
         np.asarray(Bm[0:128]), np.asarray(Bm[128:256]), np.asarray(W2p),
         K2b],
        axis=1,
    ).astype(ml_dtypes.bfloat16)
    return np.ascontiguousarray(PK), Wt


def kernel(X, centers, sigmas, W, b):
    X = np.asarray(X, dtype=np.float32)
    centers = np.asarray(centers, dtype=np.float32)
    sigmas = np.asarray(sigmas, dtype=np.float32)
    W = np.asarray(W, dtype=np.float32)
    b = np.asarray(b, dtype=np.float32)

    PK, Wt = _host_prep(centers, sigmas, W, b)
    Xb = X.astype(ml_dtypes.bfloat16)
    nc = _get_nc()
    in_maps = [
        {
            "X": np.ascontiguousarray(Xb[k * BL:(k + 1) * BL]),
            "PK": PK, "Wt": Wt,
        }
        for k in range(NCORES)
    ]
    res = bass_utils.run_bass_kernel_spmd(nc, in_maps, core_ids=list(range(NCORES)))
    return np.concatenate([res.results[k]["out"] for k in range(NCORES)], axis=0)
